# revision 1
# baseline (speedup 1.0000x reference)
"""Trainium2 Bass kernel for the masked depth-binned 3x3 conv (Conv2.5D).

Contract: kernel(**inputs) takes the FULL numpy inputs
  x     [8, 128, 64, 64] f32
  depth [8, 1, 64, 64]   f32
  fx    [8]              f32
  w0/w1/w2 [128, 128, 3, 3] f32
and returns the full output [8, 128, 64, 64] f32.

Strategy: data-parallel over N across the 8 NeuronCores (one sample per
core). Per core the op is decomposed as 27 shifted 1x1 matmuls (3 depth
bins x 9 taps) accumulated in PSUM. The depth-bin masks are computed
on-device in exact f32 in a compact [64,64] layout, packed into a
per-tap selector code (1/2/4, bins are disjoint), broadcast across the
128 partitions by DMA, and applied to the (padded, fp16) activations
with one fused is_equal+mult DVE op per (bin, tap).
"""

import numpy as np

import concourse.bass as bass
import concourse.mybir as mybir
import concourse.bacc as bacc
import concourse.tile as tile
from concourse.bass_utils import run_bass_kernel_spmd

F32 = mybir.dt.float32
F16 = mybir.dt.float16
AF = mybir.ActivationFunctionType
OP = mybir.AluOpType

N, C, O, H, W = 8, 128, 128, 64, 64
L = H * W                    # 4096
PAD = 66                     # padded image row stride (66x66 image)
LP = PAD * PAD               # 4356
NT = 8                       # number of 512-wide output column tiles
NTW = L // NT                # 512
CODES = (1.0, 2.0, 4.0)      # selector code per branch b0/b1/b2


def _build_program(loop_n=None, ablate=()):
    """loop_n: if set, wrap the whole per-sample body in an on-device
    For_i loop (used only for timing measurements).
    ablate: timing-diagnostic switches ("bcast", "act", "mult", "mm")
    that remove pieces of the pipeline (results become wrong)."""
    nc = bacc.Bacc("TRN2", target_bir_lowering=False, debug=False)
    for cval in (-1.0, -0.5):
        cten = nc.alloc_sbuf_tensor(f"const-f32-{cval}", [128, 1], F32)
        nc.gpsimd.memset(cten.ap(), cval)
        nc.const_aps.aps[(F32, cval)] = cten.ap()

    x_in = nc.dram_tensor("x_in", [C, L], F32, kind="ExternalInput")
    d_in = nc.dram_tensor("d_in", [H, W], F32, kind="ExternalInput")
    # receives 1/fx (host-computed, correctly-rounded f32)
    fx_in = nc.dram_tensor("fx_in", [1, 1], F32, kind="ExternalInput")
    w_in = nc.dram_tensor("w_in", [27, C, O], F16, kind="ExternalInput")
    out_d = nc.dram_tensor("out", [O, L], F32, kind="ExternalOutput")

    with tile.TileContext(nc) as tc:
        with (
            tc.tile_pool(name="const", bufs=1) as cpool,
            tc.tile_pool(name="work", bufs=2) as wpool,
            tc.tile_pool(name="selk", bufs=9) as skpool,
            tc.tile_pool(name="selp", bufs=2) as selpool,
            tc.tile_pool(name="rowp", bufs=3, space="DRAM") as rowpool,
            tc.tile_pool(name="masked", bufs=4) as mpool,
            tc.tile_pool(name="mbits", bufs=4) as bpool,
            tc.tile_pool(name="psum", bufs=1, space="PSUM") as ppool,
        ):
          with (tc.For_i(0, loop_n, 1)
                if loop_n is not None
                else __import__("contextlib").nullcontext()):
              # ---- load & prep -------------------------------------------------
              w_sb = cpool.tile([C, 27 * O], F16, tag="w")
              nc.sync.dma_start(
                  out=w_sb[:, :].rearrange("c (t o) -> c t o", t=27),
                  in_=w_in[:, :, :].transpose([1, 0, 2]),
              )

              fx_sb = cpool.tile([1, 1], F32, tag="fx")
              nc.sync.dma_start(out=fx_sb[:, :], in_=fx_in[:, :])
              fx_col = cpool.tile([64, 1], F32, tag="fxcol")
              nc.gpsimd.partition_broadcast(fx_col[:, :], fx_sb[:1, :])

              dpad = cpool.tile([PAD, PAD], F32, tag="dpad")
              nc.vector.memset(dpad[:, :], 0.0)
              nc.sync.dma_start(out=dpad[1:65, 1:65], in_=d_in[:, :])
              # engine ops need partition-base 0/32/64/96: DMA-copy the three
              # row-shifted views of dpad down to partition 0.
              drow = []
              for dy in range(3):
                  dr = cpool.tile([64, PAD], F32, tag=f"drow{dy}", name=f"drow{dy}")
                  nc.sync.dma_start(out=dr[:, :], in_=dpad[dy : dy + 64, :])
                  drow.append(dr)

              # padded fp16 activations; xb is xa shifted right by one element
              # so that odd-dx tap views stay 4-byte aligned (DVE 2x mode).
              xa = cpool.tile([C, LP], F16, tag="xa")
              xb = cpool.tile([C, LP + 1], F16, tag="xb")
              xa_r = xa[:, :].rearrange("c (r w) -> c r w", w=PAD)
              # zero only the padding border (interior is overwritten by the
              # casting DMA below)
              nc.vector.memset(xa[:, 0:PAD], 0.0)             # top row
              nc.vector.memset(xa[:, LP - PAD : LP], 0.0)     # bottom row
              nc.vector.memset(xa_r[:, 1:65, 0:1], 0.0)       # left col
              nc.vector.memset(xa_r[:, 1:65, 65:66], 0.0)     # right col
              # casting DMA (f32 dram -> fp16 sbuf)
              nc.gpsimd.dma_start(
                  out=xa_r[:, 1:65, 1:65],
                  in_=x_in[:, :].rearrange("c (h w) -> c h w", w=W),
              )
              nc.vector.memset(xb[:, 0:1], 0.0)
              nc.vector.tensor_copy(xb[:, 1 : LP + 1], xa[:, :])
              xb_r = xb[:, 1 : LP + 1].rearrange("c (r w) -> c r w", w=PAD)

              # ---- mask precursors (exact f32) --------------------------------
              cview = drow[1][:, 1:65]                      # center depth [64,64]
              g = wpool.tile([64, 64], F32, tag="g")
              h = wpool.tile([64, 64], F32, tag="h")
              t0 = wpool.tile([64, 64], F32, tag="t0")
              t2 = wpool.tile([64, 64], F32, tag="t2")
              nc.vector.tensor_scalar(
                  out=g[:, :], in0=cview, scalar1=fx_col[:, :], scalar2=None,
                  op0=OP.mult,
              )
              nc.vector.tensor_scalar(
                  out=h[:, :], in0=g[:, :], scalar1=0.5, scalar2=None, op0=OP.mult
              )
              hneg = wpool.tile([64, 64], F32, tag="hneg")
              nc.vector.tensor_scalar(
                  out=hneg[:, :], in0=h[:, :], scalar1=-1.0, scalar2=None, op0=OP.mult
              )
              nc.vector.tensor_tensor(out=t0[:, :], in0=cview, in1=g[:, :], op=OP.add)
              nc.vector.tensor_tensor(out=t2[:, :], in0=cview, in1=g[:, :], op=OP.subtract)

              # ---- main loop (tap-major): selector -> 3 masked rhs -> matmuls -
              nt_eff = 1 if "mm" in ablate else NT
              psums = [
                  ppool.tile([O, NTW], F32, tag=f"ps{t}", name=f"ps{t}")
                  for t in range(nt_eff)
              ]
              for dy in range(3):
                row3 = rowpool.tile([3, L], F16, tag="selrow")
                sel3 = selpool.tile([C, 3 * L], F16, tag="sel")
                for dx in range(3):
                  k = dy * 3 + dx
                  dk = drow[dy][:, dx : dx + 64]
                  u = wpool.tile([64, 64], F32, tag="u")
                  m0 = wpool.tile([64, 64], F32, tag="m0")
                  m1 = wpool.tile([64, 64], F32, tag="m1")
                  m2 = wpool.tile([64, 64], F32, tag="m2")
                  sel01 = wpool.tile([64, 64], F32, tag="sel01")
                  if dx == 0:
                      selk3 = skpool.tile([64, 192], F16, tag="selk3")
                  selk = selk3[:, dx * 64 : (dx + 1) * 64]
                  c2t = wpool.tile([64, 64], F32, tag="c2t")
                  for tgt, m in ((t0, m0), (None, m1), (t2, m2)):
                      tv = cview if tgt is None else tgt[:, :]
                      # m = (|d - t| <= h)  ==  (u <= h) & (u >= -h), u = d - t
                      nc.vector.tensor_tensor(out=u[:, :], in0=dk, in1=tv, op=OP.subtract)
                      nc.vector.tensor_tensor(out=m[:, :], in0=u[:, :], in1=h[:, :], op=OP.is_le)
                      nc.vector.tensor_tensor(
                          out=c2t[:, :], in0=u[:, :], in1=hneg[:, :], op=OP.is_ge
                      )
                      nc.vector.tensor_tensor(
                          out=m[:, :], in0=m[:, :], in1=c2t[:, :], op=OP.logical_and
                      )
                  # SEL = 2*(m1 - m0 + 3*m2): codes none:0 b0:-2 b1:2 b2:6
                  nc.vector.tensor_tensor(
                      out=sel01[:, :], in0=m1[:, :], in1=m0[:, :], op=OP.subtract
                  )
                  nc.vector.scalar_tensor_tensor(
                      out=sel01[:, :], in0=m2[:, :], scalar=3.0, in1=sel01[:, :],
                      op0=OP.mult, op1=OP.add,
                  )
                  nc.vector.tensor_scalar(
                      out=selk[:, :], in0=sel01[:, :], scalar1=2.0, scalar2=None,
                      op0=OP.mult,
                  )
                  if dx == 2:
                      if "bcast" in ablate:
                          if dy == 0:
                              nc.vector.memset(sel3[:, :], 2.0)
                              sel3_keep = sel3
                          sel3 = sel3_keep
                      else:
                          # flatten all 3 taps: [64, 3*64] sbuf -> [3, L] dram
                          # (dram side iterated in (p, t, x) order to match
                          # the sbuf partition-major AP)
                          nc.sync.dma_start(
                              out=bass.AP(
                                  row3.tensor,
                                  row3[:, :].offset,
                                  [[64, 64], [L, 3], [1, 64]],
                              ),
                              in_=selk3[:, :].rearrange("p (t x) -> p t x", x=64),
                          )
                          # per-tap broadcasts on alternating DGE queues
                          # (SP / ACT HWDGE + 3 gpsimd SWDGE queues)
                          for t in range(3):
                              eng = (nc.sync, nc.scalar, nc.gpsimd)[(dy + t) % 3]
                              eng.dma_start(
                                  out=sel3[:, t * L : (t + 1) * L],
                                  in_=row3[t : t + 1, :].partition_broadcast(C),
                              )

                for dx in range(3):
                  k = dy * 3 + dx
                  sel_k = sel3[:, dx * L : (dx + 1) * L]
                  xsrc = xa_r if dx % 2 == 0 else xb_r
                  xview = xsrc[:, dy : dy + 64, dx : dx + 64]
                  for b in range(3):
                      bk = k * 3 + b
                      # branch mask (1.0/0.0): DVE tensor_scalar is_equal
                      # runs in 4x mode (single-src fp16 SBUF); alternate
                      # taps build it on ACT via Relu(1 - |SEL - code|) to
                      # offload the DVE.
                      mbit = bpool.tile([C, L], F16, tag="mb")
                      if "act" in ablate:
                          mbit = sel_k
                      elif b == 0:
                          # m0 = relu(-SEL/2): 1 iff SEL == -2
                          nc.scalar.activation(
                              out=mbit[:, :], in_=sel_k[:, :], func=AF.Relu,
                              bias=0.0, scale=-0.5,
                          )
                      elif b == 1:
                          # m1 = (SEL == 2): DVE tensor_scalar in 4x mode
                          nc.vector.tensor_scalar(
                              out=mbit[:, :], in0=sel_k[:, :], scalar1=2.0,
                              scalar2=None, op0=OP.is_equal,
                          )
                      else:
                          # m2 = relu(SEL/4 - 1/2): 1 iff SEL == 6
                          nc.scalar.activation(
                              out=mbit[:, :], in_=sel_k[:, :], func=AF.Relu,
                              bias=-0.5, scale=0.25,
                          )
                      masked = mpool.tile([C, L], F16, tag="mx")
                      if "mult" in ablate:
                          masked = mbit
                      else:
                          nc.vector.tensor_tensor(
                              out=masked[:, :].rearrange("c (h w) -> c h w", w=W),
                              in0=mbit[:, :].rearrange("c (h w) -> c h w", w=W),
                              in1=xview,
                              op=OP.mult,
                          )
                      for t in range(nt_eff):
                          nc.tensor.matmul(
                              psums[t][:, :],
                              w_sb[:, bk * O : (bk + 1) * O],
                              masked[:, t * NTW : (t + 1) * NTW],
                              start=(bk == 0),
                              stop=(bk == 26),
                          )

              # ---- evict ------------------------------------------------------
              osb = cpool.tile([O, L], F32, tag="osb")
              for t in range(nt_eff):
                  nc.scalar.activation(
                      out=osb[:, t * NTW : (t + 1) * NTW],
                      in_=psums[t][:, :],
                      func=AF.Copy,
                  )
              nc.sync.dma_start(out=out_d[:, :], in_=osb[:, :])

    nc.compile()
    return nc


_NC = None


def _get_program():
    global _NC
    if _NC is None:
        _NC = _build_program()
    return _NC


def _prep_weights(w0, w1, w2):
    # wt[k*3 + b] = w_b[:, :, k//3, k%3].T  -> [27, C(K), O(M)] fp16
    wt = np.empty((27, C, O), np.float32)
    for b, w in enumerate((w0, w1, w2)):
        wt[b::3] = w.reshape(O, C, 9).transpose(2, 1, 0)
    return wt.astype(np.float16)


def kernel(**inputs):
    x = np.ascontiguousarray(inputs["x"], np.float32)
    depth = np.ascontiguousarray(inputs["depth"], np.float32)
    fx = np.ascontiguousarray(inputs["fx"], np.float32)
    wt = _prep_weights(
        np.asarray(inputs["w0"], np.float32),
        np.asarray(inputs["w1"], np.float32),
        np.asarray(inputs["w2"], np.float32),
    )

    nc = _get_program()
    in_maps = []
    for i in range(N):
        in_maps.append(
            {
                "x_in": np.ascontiguousarray(x[i].reshape(C, L)),
                "d_in": np.ascontiguousarray(depth[i, 0]),
                "fx_in": (np.float32(1.0) / fx[i]).reshape(1, 1),
                "w_in": wt,
            }
        )
    res = run_bass_kernel_spmd(nc, in_maps, core_ids=list(range(N)))
    out = np.stack([res.results[i]["out"] for i in range(N)])
    return out.reshape(N, O, H, W).astype(np.float32)



# revision 4
# speedup vs baseline: 1.5085x; 1.5085x over previous
"""Trainium2 Bass kernel for the masked depth-binned 3x3 conv (Conv2.5D).

Contract: kernel(**inputs) takes the FULL numpy inputs
  x     [8, 128, 64, 64] f32
  depth [8, 1, 64, 64]   f32
  fx    [8]              f32
  w0/w1/w2 [128, 128, 3, 3] f32
and returns the full output [8, 128, 64, 64] f32.

Strategy: data-parallel over N across the 8 NeuronCores (one sample per
core). Per core the op is decomposed as shifted 1x1 matmuls accumulated
in PSUM, with the 3 depth bins folded into a Vandermonde "moments"
basis: per tap a single selector field T in {0, 1, -1, 2} (branch codes
t_b = 1/-1/2, none=0) is broadcast across partitions, and the three
matmul rhs operands are the exact fp16 moments u_j = x * T^j
(j = 1..3, power-of-2 codes so the multiplies are exact). The three
branch weight matrices are combined on the host into V_j = sum_b
inv(Vandermonde)[j,b] * W_b so that sum_j V_j @ u_j == sum_b W_b @
(x * m_b) wherever at most one mask is active (verified disjoint for
this input; padding taps have x = 0 so their codes are don't-care).
The center tap is always branch 1 (|d-c| = 0 <= h), so it skips
masking entirely and contributes one plain matmul of x.

This removes all per-(tap,branch) mask decode work (the old ACT Relu /
DVE is_equal ops) and cuts the big DVE multiplies from 27+9 ops to 24,
leaving DVE ~55us as the critical engine next to PE ~43us.
"""

import numpy as np

import concourse.bass as bass
import concourse.mybir as mybir
import concourse.bacc as bacc
import concourse.tile as tile
from concourse.bass_utils import run_bass_kernel_spmd

F32 = mybir.dt.float32
F16 = mybir.dt.float16
AF = mybir.ActivationFunctionType
OP = mybir.AluOpType

N, C, O, H, W = 8, 128, 128, 64, 64
L = H * W                    # 4096
PAD = 66                     # padded image row stride (66x66 image)
LP = PAD * PAD               # 4356
NT = 8                       # number of 512-wide output column tiles
NTW = L // NT                # 512
KS = (0, 1, 2, 3, 5, 6, 7, 8)  # off-center taps, processing order
NMM = 1 + 3 * len(KS)        # accumulation group length (center + moments)


def _build_program(loop_n=None, ablate=()):
    """loop_n: if set, wrap the whole per-sample body in an on-device
    For_i loop (used only for timing measurements).
    ablate: timing-diagnostic switches ("bcast", "mult", "mm", "prec")
    that remove pieces of the pipeline (results become wrong)."""
    nc = bacc.Bacc("TRN2", target_bir_lowering=False, debug=False)
    for cval in (-1.0, -0.5):
        cten = nc.alloc_sbuf_tensor(f"const-f32-{cval}", [128, 1], F32)
        nc.gpsimd.memset(cten.ap(), cval)
        nc.const_aps.aps[(F32, cval)] = cten.ap()

    x_in = nc.dram_tensor("x_in", [C, L], F32, kind="ExternalInput")
    d_in = nc.dram_tensor("d_in", [H, W], F32, kind="ExternalInput")
    # receives 1/fx (host-computed, correctly-rounded f32)
    fx_in = nc.dram_tensor("fx_in", [1, 1], F32, kind="ExternalInput")
    w_in = nc.dram_tensor("w_in", [NMM, C, O], F16, kind="ExternalInput")
    out_d = nc.dram_tensor("out", [O, L], F32, kind="ExternalOutput")

    with tile.TileContext(nc) as tc:
        with (
            tc.tile_pool(name="const", bufs=1) as cpool,
            tc.tile_pool(name="xabuf", bufs=2) as xpool,
            tc.tile_pool(name="work", bufs=2) as wpool,
            tc.tile_pool(name="selk", bufs=2) as skpool,
            tc.tile_pool(name="selp", bufs=3) as selpool,
            tc.tile_pool(name="rowp", bufs=2, space="DRAM") as rowpool,
            tc.tile_pool(name="masked", bufs=6) as mpool,
            tc.tile_pool(name="psum", bufs=1, space="PSUM") as ppool,
        ):
          with (tc.For_i(0, loop_n, 1)
                if loop_n is not None
                else __import__("contextlib").nullcontext()):
              # ---- load & prep -------------------------------------------------
              w_sb = cpool.tile([C, NMM * O], F16, tag="w")
              nc.sync.dma_start(
                  out=w_sb[:, :].rearrange("c (t o) -> c t o", t=NMM),
                  in_=w_in[:, :, :].transpose([1, 0, 2]),
              )

              fx_sb = cpool.tile([1, 1], F32, tag="fx")
              nc.sync.dma_start(out=fx_sb[:, :], in_=fx_in[:, :])
              fx_col = cpool.tile([64, 1], F32, tag="fxcol")
              nc.gpsimd.partition_broadcast(fx_col[:, :], fx_sb[:1, :])

              dpad = cpool.tile([PAD, PAD], F32, tag="dpad")
              nc.vector.memset(dpad[:, :], 0.0)
              nc.sync.dma_start(out=dpad[1:65, 1:65], in_=d_in[:, :])
              # engine ops need partition-base 0/32/64/96: DMA-copy the three
              # row-shifted views of dpad down to partition 0.
              drow = []
              for dy in range(3):
                  dr = cpool.tile([64, PAD], F32, tag=f"drow{dy}", name=f"drow{dy}")
                  nc.sync.dma_start(out=dr[:, :], in_=dpad[dy : dy + 64, :])
                  drow.append(dr)

              # padded fp16 activations; xb is xa shifted right by one element
              # so that odd-dx tap views stay 4-byte aligned (DVE 2x mode).
              xa = xpool.tile([C, LP], F16, tag="xa")
              xb = xpool.tile([C, LP + 1], F16, tag="xb")
              xa_r = xa[:, :].rearrange("c (r w) -> c r w", w=PAD)
              # zero only the padding border (interior is overwritten by the
              # casting DMA below)
              nc.vector.memset(xa[:, 0:PAD], 0.0)             # top row
              nc.vector.memset(xa[:, LP - PAD : LP], 0.0)     # bottom row
              nc.vector.memset(xa_r[:, 1:65, 0:1], 0.0)       # left col
              nc.vector.memset(xa_r[:, 1:65, 65:66], 0.0)     # right col
              # casting DMA (f32 dram -> fp16 sbuf)
              nc.gpsimd.dma_start(
                  out=xa_r[:, 1:65, 1:65],
                  in_=x_in[:, :].rearrange("c (h w) -> c h w", w=W),
              )
              nc.vector.memset(xb[:, 0:1], 0.0)
              nc.vector.tensor_copy(xb[:, 1 : LP + 1], xa[:, :])
              xb_r = xb[:, 1 : LP + 1].rearrange("c (r w) -> c r w", w=PAD)

              # ---- selector precursors (exact f32, all 8 taps batched) --------
              cview = drow[1][:, 1:65]                      # center depth [64,64]
              g = wpool.tile([64, 64], F32, tag="g")
              h = wpool.tile([64, 64], F32, tag="h")
              t0 = wpool.tile([64, 64], F32, tag="t0")
              t2 = wpool.tile([64, 64], F32, tag="t2")
              nc.vector.tensor_scalar(
                  out=g[:, :], in0=cview, scalar1=fx_col[:, :], scalar2=None,
                  op0=OP.mult,
              )
              nc.vector.tensor_scalar(
                  out=h[:, :], in0=g[:, :], scalar1=0.5, scalar2=None, op0=OP.mult
              )
              nc.vector.tensor_tensor(out=t0[:, :], in0=cview, in1=g[:, :], op=OP.add)
              nc.vector.tensor_tensor(out=t2[:, :], in0=cview, in1=g[:, :], op=OP.subtract)

              NK = len(KS)
              selk = skpool.tile([64, NK * 64], F16, tag="selk")
              if "prec" in ablate:
                  nc.vector.memset(selk[:, :], 1.0)
              else:
                  d8 = wpool.tile([64, NK * 64], F32, tag="d8")
                  for i, k in enumerate(KS):
                      dy, dx = k // 3, k % 3
                      nc.vector.tensor_copy(
                          d8[:, i * 64 : (i + 1) * 64], drow[dy][:, dx : dx + 64]
                      )
                  d8v = d8[:, :].rearrange("p (t x) -> p t x", x=64)
                  hneg = wpool.tile([64, 64], F32, tag="hneg")
                  nc.vector.tensor_scalar(
                      out=hneg[:, :], in0=h[:, :], scalar1=-1.0, scalar2=None,
                      op0=OP.mult,
                  )
                  h_rep = h[:, :].unsqueeze(1).broadcast_to([64, NK, 64])
                  hneg_rep = hneg[:, :].unsqueeze(1).broadcast_to([64, NK, 64])
                  u = wpool.tile([64, NK * 64], F32, tag="u")
                  uv = u[:, :].rearrange("p (t x) -> p t x", x=64)
                  c2t = wpool.tile([64, NK * 64], F32, tag="c2t")
                  c2tv = c2t[:, :].rearrange("p (t x) -> p t x", x=64)
                  m0 = wpool.tile([64, NK * 64], F32, tag="m0")
                  m1 = wpool.tile([64, NK * 64], F32, tag="m1")
                  m2 = wpool.tile([64, NK * 64], F32, tag="m2")
                  for m, tv in ((m0, t0), (m1, cview), (m2, t2)):
                      tv_rep = (
                          (tv if isinstance(tv, bass.AP) else tv[:, :])
                          .unsqueeze(1)
                          .broadcast_to([64, NK, 64])
                      )
                      mv = m[:, :].rearrange("p (t x) -> p t x", x=64)
                      # m = (u <= h) & (u >= -h), u = d - t
                      nc.vector.tensor_tensor(
                          out=uv, in0=d8v, in1=tv_rep, op=OP.subtract
                      )
                      nc.vector.tensor_tensor(
                          out=mv, in0=uv, in1=h_rep, op=OP.is_le
                      )
                      nc.vector.tensor_tensor(
                          out=c2tv, in0=uv, in1=hneg_rep, op=OP.is_ge
                      )
                      nc.vector.tensor_tensor(
                          out=m[:, :], in0=m[:, :], in1=c2t[:, :],
                          op=OP.logical_and,
                      )
                  # T = m0 - m1 + 2*m2  (codes: b0=1, b1=-1, b2=2, none=0)
                  s01 = wpool.tile([64, NK * 64], F32, tag="s01")
                  nc.vector.tensor_tensor(
                      out=s01[:, :], in0=m0[:, :], in1=m1[:, :], op=OP.subtract
                  )
                  t32 = wpool.tile([64, NK * 64], F32, tag="t32")
                  nc.vector.scalar_tensor_tensor(
                      out=t32[:, :], in0=m2[:, :], scalar=2.0, in1=s01[:, :],
                      op0=OP.mult, op1=OP.add,
                  )
                  nc.vector.tensor_copy(selk[:, :], t32[:, :])

              # pack all 8 selector planes: [64, 8*64] sbuf -> [8, L] dram
              # (dram side iterated in (p, t, x) order to match the sbuf
              # partition-major AP)
              row9 = rowpool.tile([NK, L], F16, tag="selrow")
              nc.sync.dma_start(
                  out=bass.AP(
                      row9.tensor,
                      row9[:, :].offset,
                      [[64, 64], [L, NK], [1, 64]],
                  ),
                  in_=selk[:, :].rearrange("p (t x) -> p t x", x=64),
              )

              # ---- matmul pipeline -------------------------------------------
              nt_eff = 1 if "mm" in ablate else NT
              psums = [
                  ppool.tile([O, NTW], F32, tag=f"ps{t}", name=f"ps{t}")
                  for t in range(nt_eff)
              ]
              # center tap first: always branch 1, no masking
              xc = xa_r[:, 1:65, 1:65]
              for t in range(nt_eff):
                  nc.tensor.matmul(
                      psums[t][:, :],
                      w_sb[:, 0:O],
                      xc[:, 8 * t : 8 * t + 8, :],
                      start=True,
                      stop=False,
                  )

              if "bcast" in ablate:
                  sel_const = selpool.tile([C, L], F16, tag="selc")
                  nc.vector.memset(sel_const[:, :], 1.0)

              for i, k in enumerate(KS):
                  dy, dx = k // 3, k % 3
                  if "bcast" in ablate:
                      sel_t = sel_const
                  else:
                      sel_t = selpool.tile([C, L], F16, tag="sel")
                      eng = (nc.sync, nc.scalar)[i % 2]
                      eng.dma_start(
                          out=sel_t[:, :],
                          in_=row9[i : i + 1, :].partition_broadcast(C),
                      )
                  sel_v = sel_t[:, :].rearrange("c (h w) -> c h w", w=W)
                  xsrc = xa_r if dx % 2 == 0 else xb_r
                  xview = xsrc[:, dy : dy + 64, dx : dx + 64]
                  us = []
                  prev = None
                  for j in range(3):
                      uj = mpool.tile([C, L], F16, tag="mx")
                      if "mult" in ablate:
                          uj = sel_t
                      elif j == 0:
                          nc.vector.tensor_tensor(
                              out=uj[:, :].rearrange("c (h w) -> c h w", w=W),
                              in0=sel_v,
                              in1=xview,
                              op=OP.mult,
                          )
                      else:
                          nc.vector.tensor_tensor(
                              out=uj[:, :], in0=prev[:, :], in1=sel_t[:, :],
                              op=OP.mult,
                          )
                      prev = uj
                      us.append(uj)
                  for j, uj in enumerate(us):
                      idx = 1 + 3 * i + j
                      for t in range(nt_eff):
                          nc.tensor.matmul(
                              psums[t][:, :],
                              w_sb[:, idx * O : (idx + 1) * O],
                              uj[:, t * NTW : (t + 1) * NTW],
                              start=False,
                              stop=(idx == NMM - 1),
                          )

              # ---- evict ------------------------------------------------------
              osb = cpool.tile([O, L], F32, tag="osb")
              for t in range(nt_eff):
                  nc.scalar.activation(
                      out=osb[:, t * NTW : (t + 1) * NTW],
                      in_=psums[t][:, :],
                      func=AF.Copy,
                  )
              nc.sync.dma_start(out=out_d[:, :], in_=osb[:, :])

    nc.compile()
    return nc


_NC = None


def _get_program():
    global _NC
    if _NC is None:
        _NC = _build_program()
    return _NC


def _prep_weights(w0, w1, w2):
    # Vandermonde decode for codes (1, -1, 2): V_j = sum_b inv(A)[j,b] W_b
    # with A[a][j] = t_a^(j+1). Slot 0 is the center tap (always branch 1).
    A = np.array([[1, 1, 1], [-1, 1, -1], [2, 4, 8]], np.float64)
    Cf = np.linalg.inv(A)
    ws = (np.asarray(w0, np.float64), np.asarray(w1, np.float64),
          np.asarray(w2, np.float64))
    V = [sum(Cf[j, b] * ws[b] for b in range(3)) for j in range(3)]  # [O,C,3,3]
    wt = np.empty((NMM, C, O), np.float32)
    wt[0] = np.asarray(w1, np.float32)[:, :, 1, 1].T
    for i, k in enumerate(KS):
        for j in range(3):
            wt[1 + 3 * i + j] = V[j][:, :, k // 3, k % 3].T
    return wt.astype(np.float16)


def kernel(**inputs):
    x = np.ascontiguousarray(inputs["x"], np.float32)
    depth = np.ascontiguousarray(inputs["depth"], np.float32)
    fx = np.ascontiguousarray(inputs["fx"], np.float32)
    wt = _prep_weights(inputs["w0"], inputs["w1"], inputs["w2"])

    nc = _get_program()
    in_maps = []
    for i in range(N):
        in_maps.append(
            {
                "x_in": np.ascontiguousarray(x[i].reshape(C, L)),
                "d_in": np.ascontiguousarray(depth[i, 0]),
                "fx_in": (np.float32(1.0) / fx[i]).reshape(1, 1),
                "w_in": wt,
            }
        )
    res = run_bass_kernel_spmd(nc, in_maps, core_ids=list(range(N)))
    out = np.stack([res.results[i]["out"] for i in range(N)])
    return out.reshape(N, O, H, W).astype(np.float32)
